# revision 1
# baseline (speedup 1.0000x reference)
"""Sliding-window GQA causal self-attention on 8 TRN2 NeuronCores.

Sharding: sequence-parallel. B=2 batches x 4 chunks of 512 tokens = 8 shards.
Sliding window (512) makes attention local: each chunk only needs the 512
preceding tokens (halo) for K/V, so there are NO collectives. Each core:
  qkv proj (bf16 matmul) -> rotate-half RoPE + RMS-norm -> banded attention
  (scores computed transposed so exp'd probs feed attn@V directly as lhsT,
   softmax denominators fused via a ones-column in V) -> output proj.
Host pre-transposes x / weights, de-interleaves RoPE pairs into the qkv
weight so RoPE is a contiguous rotate-half, and precomputes cos/sin tables
and the multiplicative band mask per core.
"""

import sys

sys.path.insert(0, "/opt/trn_rl_repo")

import numpy as np
import ml_dtypes

import concourse.bass as bass
import concourse.mybir as mybir
import concourse.tile as tile
from concourse import bacc
from concourse.bass_utils import run_bass_kernel_spmd
from concourse.masks import make_identity


def _install_ntff_hook():
    """antenv.axon_hooks is absent on this image; bridge the ctypes NTFF
    profiling hook from trn_agent_boot so trace=True works."""
    import types
    try:
        import antenv.axon_hooks  # noqa: F401
        return
    except ImportError:
        pass
    try:
        from trn_agent_boot.trn_boot import _ntff_profile_via_ctypes
        hook = _ntff_profile_via_ctypes("/opt/axon/libaxon_pjrt.so")
    except Exception:
        hook = None
    mod = types.ModuleType("antenv.axon_hooks")
    mod.get_axon_ntff_profile_hook = lambda: hook
    mod.set_axon_ntff_profile_hook = lambda h: None
    sys.modules["antenv.axon_hooks"] = mod


_install_ntff_hook()

BF16 = mybir.dt.bfloat16
F32 = mybir.dt.float32

B, T, C = 2, 2048, 2048
HQ, HKV, DH, WIN = 16, 4, 128, 512
RMS_EPS = 1.1920929e-07
CHUNK = 512          # own tokens per core
WTOK = 1024          # window tokens per core (halo 512 + own 512)
NKT = WTOK // 128    # 8 k-tiles
NQT = CHUNK // 128   # 4 q-tiles
KWIN = 640           # keys per q-tile (5 blocks of 128)
SCALE = 1.0 / np.sqrt(DH)

_NC_CACHE = [None]


def _build_nc():
    nc = bacc.Bacc("TRN2", target_bir_lowering=False, debug=False, num_devices=8)

    xt = nc.declare_dram_parameter("xt", [C, WTOK], BF16, False)        # x^T window
    wq = nc.declare_dram_parameter("wq", [C, (HQ + 2 * HKV) * DH], BF16, False)
    wp = nc.declare_dram_parameter("wp", [C, C], BF16, False)
    csq = nc.declare_dram_parameter("csq", [CHUNK, HQ * DH], BF16, False)
    ssq = nc.declare_dram_parameter("ssq", [CHUNK, HQ * DH], BF16, False)
    csk = nc.declare_dram_parameter("csk", [WTOK, HKV * DH], BF16, False)
    ssk = nc.declare_dram_parameter("ssk", [WTOK, HKV * DH], BF16, False)
    mask = nc.declare_dram_parameter("mask", [NQT, 128, 256], BF16, False)
    vld = nc.declare_dram_parameter("vld", [WTOK, 1], BF16, False)
    out = nc.declare_dram_parameter("out", [CHUNK, C], F32, True)

    xt_r = xt.ap().rearrange("(k p) t -> k p t", p=128)       # 16 x [128,1024]
    csq_r = csq.ap().rearrange("(n p) d -> n p d", p=128)     # 4 x [128,2048]
    ssq_r = ssq.ap().rearrange("(n p) d -> n p d", p=128)
    csk_r = csk.ap().rearrange("(n p) d -> n p d", p=128)     # 8 x [128,512]
    ssk_r = ssk.ap().rearrange("(n p) d -> n p d", p=128)
    mask_r = mask.ap()

    with tile.TileContext(nc) as tc:
        with (
            tc.tile_pool(name="singles", bufs=1) as singles,
            tc.tile_pool(name="wtiles", bufs=2) as wtiles,      # streamed weight cols
            tc.tile_pool(name="raw", bufs=1) as raw,            # Q/K raw + tables
            tc.tile_pool(name="work", bufs=2) as work,          # small DVE temps
            tc.tile_pool(name="ptile", bufs=4) as ptile,        # exp'd probs
            tc.tile_pool(name="psA", bufs=2, space="PSUM") as psA,    # scores (2 banks)
            tc.tile_pool(name="psB", bufs=2, space="PSUM") as psB,    # proj + AV
        ):
            ident = singles.tile([128, 128], BF16)
            make_identity(nc, ident)
            eps_t = singles.tile([128, 1], F32)
            nc.vector.memset(eps_t, RMS_EPS)

            # resident x^T: 16 tiles [128, 1024] (C-tile, tok); DMAs
            # interleaved with the first weight column inside proj_col(0)
            xts = [singles.tile([128, WTOK], BF16, name=f"xts{k}") for k in range(16)]

            # triangle masks for j=0 / j=4 only, [128, 4, 2, 128]
            msk = singles.tile([128, NQT, 2, 128], BF16)
            for i in range(NQT):
                nc.gpsimd.dma_start(out=msk[:, i, :, :].rearrange("p a b -> p (a b)"), in_=mask_r[i])

            # raw (pre-rope) Q/K; V_ext with ones column
            qraw = [raw.tile([128, HQ, DH], BF16, name=f"qraw{i}", tag=f"qraw{i}") for i in range(NQT)]
            kraw = [raw.tile([128, HKV, DH], BF16, name=f"kraw{j}", tag=f"kraw{j}") for j in range(NKT)]
            vext = [raw.tile([128, HKV, 132], BF16, name=f"vext{j}", tag=f"vext{j}") for j in range(NKT)]
            vld_r = vld.ap().rearrange("(n p) o -> n p o", p=128)
            for j in range(NKT):
                for g in range(HKV):
                    nc.gpsimd.dma_start(out=vext[j][:, g, 128:129], in_=vld_r[j])

            # ---------------- QKV projection ----------------
            # feature columns: n=0..3 Q (heads 4n..4n+3), n=4 K, n=5 V
            def proj_col(n, wsrc):
                wk = [wtiles.tile([128, 512], BF16, name=f"w{n}_{k}", tag=f"w{k}")
                      for k in range(16)]
                for k in range(16):
                    nc.sync.dma_start(
                        out=wk[k], in_=wsrc.ap()[k * 128:(k + 1) * 128,
                                                 n * 512:(n + 1) * 512])
                    if n == 0:
                        nc.sync.dma_start(out=xts[k], in_=xt_r[k])
                itiles = range(4, 8) if n < 4 else range(8)
                for i in itiles:
                    pp = psB.tile([128, 512], F32, tag="pp", name="pp")
                    for k in range(16):
                        nc.tensor.matmul(pp, xts[k][:, i * 128:(i + 1) * 128],
                                         wk[k],
                                         start=(k == 0), stop=(k == 15))
                    if n < 4:
                        qi = i - 4
                        nc.scalar.copy(
                            out=qraw[qi][:, 4 * n:4 * n + 4, :].rearrange("p a b -> p (a b)"),
                            in_=pp[:])
                    elif n == 4:
                        nc.scalar.copy(
                            out=kraw[i][:, :, :].rearrange("p a b -> p (a b)"),
                            in_=pp[:])
                    else:
                        for g in range(HKV):
                            nc.scalar.copy(out=vext[i][:, g, 0:128],
                                           in_=pp[:, g * 128:(g + 1) * 128])

            # ---------------- RoPE + RMS-norm + transpose ----------------
            # qt[d, tok] tiles: [128, (h, qi, 128)] ; kt: [128, (g, j, 128)]
            qt = singles.tile([128, HQ, NQT, 128], BF16)
            kt = singles.tile([128, HKV, NKT, 128], BF16)

            def rope_rms(src, n_heads, cs_r_, ss_r_, row, dst, dcol):
                # src: [128, H, 128] bf16 raw; writes transposed dst[:, :, dcol, :]
                h = n_heads
                sq = work.tile([128, h, DH], BF16, tag="sq")
                nc.vector.tensor_mul(sq, src, src)
                ssum = work.tile([128, h], F32, tag="ssum")
                nc.vector.reduce_sum(out=ssum, in_=sq, axis=mybir.AxisListType.X)
                rstd = work.tile([128, h], F32, tag="rstd")
                nc.scalar.activation(rstd, ssum, mybir.ActivationFunctionType.Sqrt,
                                     bias=eps_t[:], scale=1.0 / DH)
                rinv = work.tile([128, h], F32, tag="rinv")
                nc.vector.reciprocal(rinv, rstd)

                cs = work.tile([128, h, DH], BF16, tag="cs")
                ss = work.tile([128, h, DH], BF16, tag="ss")
                nc.gpsimd.dma_start(out=cs.rearrange("p a b -> p (a b)"), in_=cs_r_[row])
                nc.gpsimd.dma_start(out=ss.rearrange("p a b -> p (a b)"), in_=ss_r_[row])

                tmp = work.tile([128, h, DH], BF16, tag="tmp")
                ro = work.tile([128, h, DH], BF16, tag="ro")
                nc.vector.tensor_mul(tmp[:, :, 0:64], src[:, :, 64:128], ss[:, :, 0:64])
                nc.vector.tensor_mul(tmp[:, :, 64:128], src[:, :, 0:64], ss[:, :, 64:128])
                nc.vector.tensor_mul(ro, src, cs)
                nc.vector.tensor_add(ro, ro, tmp)
                for hh in range(h):
                    nc.vector.tensor_scalar_mul(ro[:, hh, :], ro[:, hh, :],
                                                rinv[:, hh:hh + 1])
                for hh in range(h):
                    tp = psB.tile([128, 128], BF16, tag="tp", name="tp")
                    nc.tensor.transpose(tp, ro[:, hh, :], ident)
                    nc.scalar.copy(out=dst[:, hh, dcol, :], in_=tp)

            for n in range(4):
                proj_col(n, wq)
            for qi in range(NQT):
                rope_rms(qraw[qi], HQ, csq_r, ssq_r, qi, qt, qi)
            proj_col(4, wq)
            for j in range(NKT):
                rope_rms(kraw[j], HKV, csk_r, ssk_r, j, kt, j)
            proj_col(5, wq)

            # ---------------- attention ----------------
            # yt[d, tok]: [128, (h, qi, 128)] bf16
            yt = singles.tile([128, HQ, NQT, 128], BF16)
            for h in range(HQ):
                g = h // 4
                for qi in range(NQT):
                    sc = psA.tile([128, 5, 128], F32, tag="sc", bufs=1)
                    for j in range(5):
                        nc.tensor.matmul(sc[:, j, :], kt[:, g, qi + j, :],
                                         qt[:, h, qi, :], start=True, stop=True)
                    pe = ptile.tile([128, 5, 128], BF16, tag="pe")
                    nc.scalar.activation(pe.rearrange("p a b -> p (a b)"),
                                         sc.rearrange("p a b -> p (a b)"),
                                         mybir.ActivationFunctionType.Exp,
                                         scale=float(SCALE))
                    nc.vector.tensor_mul(pe[:, 0, :], pe[:, 0, :], msk[:, qi, 0, :])
                    nc.vector.tensor_mul(pe[:, 4, :], pe[:, 4, :], msk[:, qi, 1, :])
                    yv = psB.tile([128, 132], F32, tag="yv")
                    for j in range(5):
                        nc.tensor.matmul(yv[:, 0:129], pe[:, j, :],
                                         vext[qi + j][:, g, 0:129],
                                         start=(j == 0), stop=(j == 4))
                    linv = work.tile([128, 1], F32, tag="linv", bufs=6)
                    nc.vector.reciprocal(linv, yv[:, 128:129])
                    ysb = work.tile([128, 128], BF16, tag="ysb", bufs=6)
                    nc.vector.tensor_scalar_mul(ysb, yv[:, 0:128], linv)
                    tp2 = psB.tile([128, 128], BF16, tag="tp", name="tp2")
                    nc.tensor.transpose(tp2, ysb, ident)
                    nc.scalar.copy(out=yt[:, h, qi, :], in_=tp2)

            # ---------------- output projection ----------------
            for n in range(4):
                wpc = [wtiles.tile([128, 512], BF16, name=f"wp{n}_{k}", tag=f"w{k}")
                       for k in range(16)]
                for k in range(16):
                    nc.sync.dma_start(
                        out=wpc[k], in_=wp.ap()[k * 128:(k + 1) * 128,
                                                n * 512:(n + 1) * 512])
                for i in range(NQT):
                    po = psB.tile([128, 512], F32, tag="pp", name="po")
                    for h in range(HQ):
                        nc.tensor.matmul(po, yt[:, h, i, :], wpc[h], 
                                         start=(h == 0), stop=(h == 15))
                    osb = work.tile([128, 512], F32, tag="osb")
                    nc.scalar.copy(out=osb, in_=po)
                    nc.sync.dma_start(
                        out=out.ap()[i * 128:(i + 1) * 128, n * 512:(n + 1) * 512],
                        in_=osb)
    nc.compile()
    return nc


def _host_prep(x, w_qkv, w_proj):
    """Per-core input maps (numpy, bf16)."""
    bf = ml_dtypes.bfloat16
    # de-interleave perm within each Q/K head: new j <- old sigma(j)
    sig = np.empty(DH, np.int64)
    sig[:64] = np.arange(64) * 2
    sig[64:] = np.arange(64) * 2 + 1
    wqp = w_qkv.copy()
    for h in range(HQ + HKV):          # Q heads then K heads share layout
        base = h * DH
        wqp[base:base + DH] = w_qkv[base + sig]
    wqT = np.ascontiguousarray(wqp.T).astype(bf)          # [C, 3072]
    wpT = np.ascontiguousarray(w_proj.T).astype(bf)       # [C, C]

    inv_freq = 1.0 / (10000.0 ** (np.arange(0, DH, 2, dtype=np.float64) / DH))

    def tables(pos):  # pos [n] -> cos/sin [n, DH] (rotate-half, sign-baked sin)
        f = pos[:, None].astype(np.float64) * inv_freq[None, :]
        cs = np.concatenate([np.cos(f), np.cos(f)], axis=1)
        ss = np.concatenate([-np.sin(f), np.sin(f)], axis=1)
        return cs.astype(np.float32), ss.astype(np.float32)

    in_maps = []
    for core in range(8):
        b, c = divmod(core, 4)
        own0 = c * CHUNK
        w0 = own0 - CHUNK                     # window start (may be negative)
        xw = np.zeros((WTOK, C), np.float32)
        lo = max(0, w0)
        xw[lo - w0:, :] = x[b, lo:own0 + CHUNK, :]
        xtw = np.ascontiguousarray(xw.T).astype(bf)       # [C, 1024]

        kpos = np.maximum(np.arange(w0, own0 + CHUNK), 0)
        cskv, sskv = tables(kpos)                          # [1024, 128]
        csq1, ssq1 = tables(np.arange(own0, own0 + CHUNK))  # [512, 128]

        # triangle masks (j=0, j=4) in P^T layout: [qi, key_local_p, (2, query)]
        mask = np.zeros((NQT, 128, 256), np.float32)
        for i in range(NQT):
            qp = own0 + i * 128 + np.arange(128)[:, None]
            kp = w0 + i * 128 + np.arange(KWIN)[None, :]
            m_qk = ((kp >= 0) & (kp <= qp) & (qp - kp < WIN)).astype(np.float32)
            mt = m_qk.T.reshape(5, 128, 128)      # [j, k_local, q]
            mask[i] = np.concatenate([mt[0], mt[4]], axis=1)
        vldv = (np.arange(w0, own0 + CHUNK) >= 0).astype(np.float32)[:, None]

        in_maps.append({
            "xt": xtw,
            "wq": wqT,
            "wp": wpT,
            "csq": np.tile(csq1, (1, HQ)).astype(bf),
            "ssq": np.tile(ssq1, (1, HQ)).astype(bf),
            "csk": np.tile(cskv, (1, HKV)).astype(bf),
            "ssk": np.tile(sskv, (1, HKV)).astype(bf),
            "mask": mask.astype(bf),
            "vld": vldv.astype(bf),
        })
    return in_maps


def kernel(x, w_qkv, w_proj, _trace=False):
    if _NC_CACHE[0] is None:
        _NC_CACHE[0] = _build_nc()
    nc = _NC_CACHE[0]
    in_maps = _host_prep(np.asarray(x, np.float32), np.asarray(w_qkv, np.float32),
                         np.asarray(w_proj, np.float32))
    res = run_bass_kernel_spmd(nc, in_maps, core_ids=list(range(8)), trace=_trace)
    outs = [res.results[i]["out"] for i in range(8)]
    full = np.empty((B, T, C), np.float32)
    for core in range(8):
        b, c = divmod(core, 4)
        full[b, c * CHUNK:(c + 1) * CHUNK] = outs[core]
    if _trace:
        kernel.last_exec_time_ns = res.exec_time_ns
        kernel.last_results = res
    return full

